# revision 12
# baseline (speedup 1.0000x reference)
"""Trainium2 Bass kernel for nn_BasicBlock_1w1a (binary conv BasicBlock).

Self-contained: takes FULL inputs (batch 64), shards batch across 8 NeuronCores,
runs a single SPMD Bass/Tile kernel with in-kernel AllReduces for the
training-mode BatchNorm batch statistics, gathers the full output.

Per block (twice):
  S      = conv3x3(sign(x), sign(w))        # fp8 DoubleRow matmuls, exact
  gate   = sigmoid(BN_dada(avgpool8(x) @ dw))
  u      = prelu(S * alpha * gate, a)       # fused into PSUM eviction (ACT)
  out    = BN(u) * g + b + x                # batch stats via AllReduce
"""
import os
import sys

sys.path.insert(0, "/opt/trn_rl_repo")

import numpy as np
import ml_dtypes

import concourse.bass as bass
import concourse.bacc as bacc
import concourse.tile as tile
import concourse.mybir as mybir
from concourse import bass_utils

P = 128
CI = 2
NIMG = 8
NCORES = 8
H = W = 32
S = H * W
SP = 34 * 34
CH = 2
EPS = 1e-5
MAGIC = 0x5F3759DF
AF = mybir.ActivationFunctionType
ALU = mybir.AluOpType
DT = mybir.dt
X_AXIS = mybir.AxisListType.X
XY_AXIS = mybir.AxisListType.XY

DEBUG = False
_CACHE = {}


def _build(debug=False):
    nc = bacc.Bacc("TRN2", target_bir_lowering=False, debug=False,
                   num_devices=NCORES)

    x_in = nc.dram_tensor("x", [NIMG, 256, S], DT.float32, kind="ExternalInput")
    w1_in = nc.dram_tensor("w1sb", [P, CI, 9, 2, P], DT.float8e4,
                           kind="ExternalInput")
    w2_in = nc.dram_tensor("w2sb", [P, CI, 9, 2, P], DT.float8e4,
                           kind="ExternalInput")
    # dada weights split hi/lo bf16: [c_lo, ci, hilo, oi, o_lo]
    dw1_in = nc.dram_tensor("dwt1", [P, CI, 2, 2, P], DT.bfloat16,
                            kind="ExternalInput")
    dw2_in = nc.dram_tensor("dwt2", [P, CI, 2, 2, P], DT.bfloat16,
                            kind="ExternalInput")
    # packed per-channel params: j = 0:alpha 1:a 2:g 3:b 4:dg 5:db -> [P, 6, CI]
    pk1_in = nc.dram_tensor("pk1", [P, 6, CI], DT.float32, kind="ExternalInput")
    pk2_in = nc.dram_tensor("pk2", [P, 6, CI], DT.float32, kind="ExternalInput")
    out_t = nc.dram_tensor("out", [NIMG, 256, S], DT.float32,
                           kind="ExternalOutput")

    dbg = {}
    if debug:
        dbg["u1"] = nc.dram_tensor("dbg_u1", [P, 2, NIMG, S], DT.float32,
                                   kind="ExternalOutput")
        dbg["gate1"] = nc.dram_tensor("dbg_gate1", [P, 2, NIMG], DT.float32,
                                      kind="ExternalOutput")
        dbg["p1"] = nc.dram_tensor("dbg_p1", [P, CI, NIMG, 16], DT.float32,
                                   kind="ExternalOutput")
        dbg["ar1"] = nc.dram_tensor("dbg_ar1", [P, 4], DT.float32,
                                    kind="ExternalOutput")
        dbg["ar2"] = nc.dram_tensor("dbg_ar2", [P, 4], DT.float32,
                                    kind="ExternalOutput")
        dbg["x1"] = nc.dram_tensor("dbg_x1", [NIMG, 256, S], DT.float32,
                                   kind="ExternalOutput")

    with tile.TileContext(nc) as tc:
        with tc.tile_pool(name="big", bufs=1) as big, \
             tc.tile_pool(name="small", bufs=1) as small, \
             tc.tile_pool(name="psum", bufs=3, space="PSUM") as psum_pool, \
             tc.tile_pool(name="psum_y", bufs=2, space="PSUM") as psum_y_pool, \
             tc.tile_pool(name="sq", bufs=2) as sqpool, \
             tc.tile_pool(name="tmp", bufs=2) as tmppool, \
             tc.tile_pool(name="poola", bufs=2) as poola_pool, \
             tc.tile_pool(name="dram", bufs=1, space="DRAM") as dram:

            # ---- warmup collective: absorbs ncfw init + SPMD launch skew ----
            wu = small.tile([P, 1], DT.float32, tag="wu")
            nc.gpsimd.memset(wu[:], 1.0)
            wu_i = dram.tile([P, 1], DT.float32, tag="wu_i")
            wu_o = dram.tile([P * NCORES, 1], DT.float32, tag="wu_o")
            nc.sync.dma_start(wu_i[:], wu[:])
            nc.gpsimd.collective_compute(
                "AllGather", ALU.bypass, replica_groups=[list(range(NCORES))],
                ins=[wu_i[:].opt()], outs=[wu_o[:].opt()])

            def allreduce_stats(stat_sb, out_sb, widx, name):
                """AllGather [128,4] partials + deterministic local reduce
                (AG floor ~5us vs AR ~25us)."""
                bi = dram.tile([P, 4], DT.float32, tag=f"bi_{name}{widx}")
                bo = dram.tile([P * NCORES, 4], DT.float32,
                               tag=f"bo_{name}{widx}")
                nc.sync.dma_start(bi[:], stat_sb[:])
                nc.gpsimd.collective_compute(
                    "AllGather", ALU.bypass,
                    replica_groups=[list(range(NCORES))],
                    ins=[bi[:].opt()], outs=[bo[:].opt()])
                gath = small.tile([P, NCORES, 4], DT.float32,
                                  tag=f"gth_{name}{widx}")
                nc.sync.dma_start(
                    gath[:], bo[:].rearrange("(r p) c -> p r c", p=P))
                nc.vector.tensor_reduce(out_sb[:],
                                        gath[:].rearrange("p r c -> p c r"),
                                        axis=X_AXIS, op=ALU.add)

            xt = big.tile([P, NIMG, CI, S], DT.float32, tag="xt")
            ut = big.tile([P, 2, NIMG, S], DT.float32, tag="ut")
            spad = big.tile([P, CI, NIMG, SP], DT.float8e4, tag="spad")
            w1sb = big.tile([P, CI, 9, 2, P], DT.float8e4, tag="w1")
            w2sb = big.tile([P, CI, 9, 2, P], DT.float8e4, tag="w2")
            dwt1 = big.tile([P, CI, 2, 2, P], DT.bfloat16, tag="dwt1")
            dwt2 = big.tile([P, CI, 2, 2, P], DT.bfloat16, tag="dwt2")
            pk1 = big.tile([P, 6, CI], DT.float32, tag="pk1")
            pk2 = big.tile([P, 6, CI], DT.float32, tag="pk2")

            nc.vector.memset(
                spad[:].rearrange("p c n s -> p (c n s)").bitcast(DT.int32), 0)
            def dma_x(n):
                xv = x_in[n].rearrange("(ci p) s -> p ci s", p=P)
                for ci in range(CI):
                    nc.sync.dma_start(xt[:, n, ci, :], xv[:, ci, :])

            for n in (0, 1):
                dma_x(n)
            nc.sync.dma_start(w1sb[:], w1_in[:])
            nc.sync.dma_start(pk1[:], pk1_in[:])
            nc.sync.dma_start(dwt1[:], dw1_in[:])
            for n in range(2, NIMG):
                dma_x(n)
            nc.sync.dma_start(w2sb[:], w2_in[:])
            nc.sync.dma_start(dwt2[:], dw2_in[:])
            nc.sync.dma_start(pk2[:], pk2_in[:])

            def sign_into_spad(n, ci):
                view = spad[:, ci, n, :].rearrange("p (r c) -> p r c", r=34)
                nc.scalar.activation(
                    view[:, 1:33, 1:33],
                    xt[:, n, ci, :].rearrange("p (h w) -> p h w", h=H),
                    AF.Sign)

            def pools_into(p_t, n, ci):
                pa = poola_pool.tile([P, H * 4], DT.float32, tag="poola",
                                     name=f"poola_{n}_{ci}")
                nc.vector.tensor_reduce(
                    pa[:],
                    xt[:, n, ci, :].rearrange("p (h pw w) -> p h pw w",
                                              h=H, pw=4),
                    axis=X_AXIS, op=ALU.add)
                nc.vector.tensor_reduce(
                    p_t[:, ci, n, :].rearrange("p (ph pw) -> p ph pw", ph=4),
                    pa[:].rearrange("p (ph hh pw) -> p ph pw hh", ph=4, hh=8),
                    axis=X_AXIS, op=ALU.add)

            def rsqrt_inplace(k, t, e1):
                """k = 1/sqrt(t), all DVE (quake seed + 3 Newton)."""
                ki = k.bitcast(DT.int32)
                nc.vector.tensor_scalar(ki, t.bitcast(DT.int32), 1, None,
                                        ALU.arith_shift_right)
                nc.vector.tensor_scalar(ki, ki, MAGIC, None, ALU.subtract)
                nc.vector.tensor_scalar(ki, ki, -1, None, ALU.mult)
                for _ in range(3):
                    nc.vector.tensor_mul(e1, k, k)
                    nc.vector.tensor_mul(e1, e1, t)
                    nc.vector.tensor_scalar(e1, e1, -0.5, 1.5, ALU.mult,
                                            ALU.add)
                    nc.vector.tensor_mul(k, k, e1)

            def conv_block(widx, wsb, dwt, pk, last):
                p_t = small.tile([P, CI, NIMG, 16], DT.float32, tag=f"p{widx}")
                ph = small.tile([P, CI, NIMG * 16], DT.bfloat16, tag=f"ph{widx}")
                pl = small.tile([P, CI, NIMG * 16], DT.bfloat16, tag=f"pl{widx}")
                ysb = small.tile([P, 2, NIMG * 16], DT.float32, tag=f"y{widx}")
                m_s = small.tile([P, 2, NIMG], DT.float32, tag=f"ms{widx}")
                m1 = small.tile([P, 2, NIMG], DT.float32, tag=f"m1{widx}")
                gate = small.tile([P, 2, NIMG], DT.float32, tag=f"g{widx}")
                ystat = small.tile([P, 4], DT.float32, tag=f"ys{widx}")
                usum = small.tile([P, 2, NIMG], DT.float32, tag=f"us{widx}")
                usq = small.tile([P, 2, NIMG], DT.float32, tag=f"uq{widx}")
                ustat = small.tile([P, 4], DT.float32, tag=f"ut{widx}")
                ar_y = small.tile([P, 4], DT.float32, tag=f"ary{widx}")
                ar_u = small.tile([P, 4], DT.float32, tag=f"aru{widx}")
                AB = small.tile([P, 2, 2], DT.float32, tag=f"ab{widx}")

                for n in range(NIMG):
                    for ci in range(CI):
                        if n < 2:
                            sign_into_spad(n, ci)
                        pools_into(p_t, n, ci)

                # hi/lo split of pool sums for exact-ish bf16 dada matmul
                nc.vector.tensor_copy(ph[:], p_t[:].rearrange("p c n s -> p c (n s)"))
                nc.vector.tensor_sub(pl[:],
                                     p_t[:].rearrange("p c n s -> p c (n s)"),
                                     ph[:])

                for oi in range(2):
                    psy = psum_y_pool.tile([P, NIMG * 16], DT.float32,
                                           tag="psy", name=f"psy{widx}_{oi}")
                    terms = [(hl, pp) for hl in range(2) for pp in (ph, pl)]
                    for ci in range(CI):
                        for ti, (hl, pp) in enumerate(terms):
                            nc.tensor.matmul(
                                psy[:], dwt[:, ci, hl, oi, :], pp[:, ci, :],
                                start=(ci == 0 and ti == 0),
                                stop=(ci == CI - 1 and ti == len(terms) - 1))
                    nc.scalar.activation(ysb[:, oi, :], psy[:], AF.Copy,
                                         accum_out=ystat[:, oi:oi + 1])
                    sq = sqpool.tile([P, 512], DT.float32, tag="sq",
                                     name=f"ysq{widx}_{oi}")
                    nc.scalar.activation(sq[:, :NIMG * 16], ysb[:, oi, :],
                                         AF.Square,
                                         accum_out=ystat[:, 2 + oi:3 + oi])
                    nc.vector.tensor_reduce(
                        m_s[:, oi, :],
                        ysb[:, oi, :].rearrange("p (n s) -> p n s", n=NIMG),
                        axis=X_AXIS, op=ALU.add)

                # stats exchange #1 (dada)
                allreduce_stats(ystat, ar_y, widx, "y")

                cnt_y = float(NCORES * NIMG * 16)
                for oi in range(2):
                    t = small.tile([P, 1], DT.float32, tag=f"t{widx}_{oi}")
                    mu = small.tile([P, 1], DT.float32, tag=f"mu{widx}_{oi}")
                    k = small.tile([P, 1], DT.float32, tag=f"k{widx}_{oi}")
                    e1 = small.tile([P, 1], DT.float32, tag=f"e{widx}_{oi}")
                    A = small.tile([P, 1], DT.float32, tag=f"A{widx}_{oi}")
                    B = small.tile([P, 1], DT.float32, tag=f"B{widx}_{oi}")
                    nc.vector.tensor_scalar(t[:], ar_y[:, 2 + oi:3 + oi],
                                            1.0 / cnt_y, EPS, ALU.mult, ALU.add)
                    nc.vector.tensor_scalar(mu[:], ar_y[:, oi:oi + 1],
                                            1.0 / cnt_y, None, ALU.mult)
                    nc.vector.tensor_mul(e1[:], mu[:], mu[:])
                    nc.vector.tensor_sub(t[:], t[:], e1[:])
                    rsqrt_inplace(k[:], t[:], e1[:])
                    nc.vector.tensor_mul(A[:], k[:], pk[:, 4, oi:oi + 1])
                    nc.vector.tensor_mul(B[:], mu[:], A[:])
                    nc.vector.tensor_sub(B[:], pk[:, 5, oi:oi + 1], B[:])
                    nc.vector.tensor_scalar(m1[:, oi, :], m_s[:, oi, :],
                                            1.0 / 16.0, None, ALU.mult)
                    sig = small.tile([P, NIMG], DT.float32,
                                     tag=f"sg{widx}_{oi}")
                    nc.scalar.activation(sig[:], m1[:, oi, :], AF.Sigmoid,
                                         bias=B[:], scale=A[:])
                    nc.vector.tensor_scalar(gate[:, oi, :], sig[:],
                                            pk[:, 0, oi:oi + 1], None, ALU.mult)

                # conv matmuls (fp8 DoubleRow, K=256 per MM) + fused evac
                for n in range(NIMG):
                    if n + 2 < NIMG:
                        for ci in range(CI):
                            sign_into_spad(n + 2, ci)
                    sview = spad[:, :, n, :].rearrange("p ci (r c) -> p ci r c",
                                                       r=34)
                    for oi in range(2):
                        ps = psum_pool.tile([P, S], DT.float32, tag="ps",
                                            name=f"ps{widx}_{n}_{oi}")
                        for kk in range(9):
                            dy, dx = kk // 3, kk % 3
                            lhsT = wsb[:, :, kk, oi, :]
                            for c2 in range(CH):
                                nc.tensor.matmul(
                                    ps[:, c2 * 512:(c2 + 1) * 512], lhsT,
                                    sview[:, :, c2 * 16 + dy:c2 * 16 + dy + 16,
                                          dx:dx + 32],
                                    start=(kk == 0), stop=(kk == 8),
                                    perf_mode=mybir.MatmulPerfMode.DoubleRow)
                        # u' = prelu(S, a): NO gate dependency — the gate
                        # (and alpha) fold into the BN affine later since
                        # prelu(g*S, a) = g*prelu(S, a) for g > 0.
                        u_sl = ut[:, oi, n, :]
                        nc.scalar.activation(
                            u_sl, ps[:], AF.Prelu,
                            alpha=pk[:, 1, oi:oi + 1],
                            accum_out=usum[:, oi, n:n + 1])
                        sq = sqpool.tile([P, S], DT.float32, tag="sq",
                                         name=f"sq{widx}_{n}_{oi}")
                        nc.scalar.activation(
                            sq[:], u_sl, AF.Square,
                            accum_out=usq[:, oi, n:n + 1])

                # main BN stats: gate-weighted sums of per-image accums
                # sum(u) = sum_n g'[n]*usum'[n], sum(u^2) = sum_n g'^2[n]*usq'[n]
                for oi in range(2):
                    us8 = small.tile([P, NIMG], DT.float32,
                                     tag=f"us8{widx}_{oi}")
                    nc.vector.tensor_mul(us8[:], usum[:, oi], gate[:, oi, :])
                    nc.vector.tensor_reduce(ustat[:, oi:oi + 1], us8[:],
                                            axis=X_AXIS, op=ALU.add)
                    uq8 = small.tile([P, NIMG], DT.float32,
                                     tag=f"uq8{widx}_{oi}")
                    g2 = small.tile([P, NIMG], DT.float32,
                                    tag=f"g2{widx}_{oi}")
                    nc.vector.tensor_mul(g2[:], gate[:, oi, :], gate[:, oi, :])
                    nc.vector.tensor_mul(uq8[:], usq[:, oi], g2[:])
                    nc.vector.tensor_reduce(ustat[:, 2 + oi:3 + oi], uq8[:],
                                            axis=X_AXIS, op=ALU.add)
                allreduce_stats(ustat, ar_u, widx, "u")

                cnt_u = float(NCORES * NIMG * S)
                for ci in range(2):
                    t = small.tile([P, 1], DT.float32, tag=f"tu{widx}_{ci}")
                    mu = small.tile([P, 1], DT.float32, tag=f"muu{widx}_{ci}")
                    k = small.tile([P, 1], DT.float32, tag=f"ku{widx}_{ci}")
                    e1 = small.tile([P, 1], DT.float32, tag=f"eu{widx}_{ci}")
                    nc.vector.tensor_scalar(t[:], ar_u[:, 2 + ci:3 + ci],
                                            1.0 / cnt_u, EPS, ALU.mult, ALU.add)
                    nc.vector.tensor_scalar(mu[:], ar_u[:, ci:ci + 1],
                                            1.0 / cnt_u, None, ALU.mult)
                    nc.vector.tensor_mul(e1[:], mu[:], mu[:])
                    nc.vector.tensor_sub(t[:], t[:], e1[:])
                    rsqrt_inplace(k[:], t[:], e1[:])
                    nc.vector.tensor_mul(AB[:, 0, ci:ci + 1], k[:],
                                         pk[:, 2, ci:ci + 1])
                    nc.vector.tensor_mul(e1[:], mu[:], AB[:, 0, ci:ci + 1])
                    nc.vector.tensor_sub(AB[:, 1, ci:ci + 1],
                                         pk[:, 3, ci:ci + 1], e1[:])

                # per-image scale gA[n] = A * g'[n] (gate folded in here)
                gA = small.tile([P, 2, NIMG], DT.float32, tag=f"ga{widx}")
                for ci in range(2):
                    nc.vector.tensor_scalar(gA[:, ci, :], gate[:, ci, :],
                                            AB[:, 0, ci:ci + 1], None, ALU.mult)

                # x_out = gA[n]*u' + B + x  (in place over xt);
                # scale+bias alternates ACT/DVE to halve the critical path
                for n in range(NIMG):
                    ov = out_t[n].rearrange("(ci p) s -> p ci s", p=P)
                    for ci in range(CI):
                        tmp = tmppool.tile([P, S], DT.float32, tag="tmp",
                                           name=f"tmp{widx}_{n}_{ci}")
                        if n % 2 == 0:
                            nc.scalar.activation(tmp[:], ut[:, ci, n, :],
                                                 AF.Identity,
                                                 bias=AB[:, 1, ci:ci + 1],
                                                 scale=gA[:, ci, n:n + 1])
                        else:
                            nc.vector.tensor_scalar(tmp[:], ut[:, ci, n, :],
                                                    gA[:, ci, n:n + 1],
                                                    AB[:, 1, ci:ci + 1],
                                                    ALU.mult, ALU.add)
                        nc.vector.tensor_add(xt[:, n, ci, :], tmp[:],
                                             xt[:, n, ci, :])
                        if last:
                            nc.sync.dma_start(ov[:, ci, :], xt[:, n, ci, :])

                if debug and widx == 1:
                    nc.sync.dma_start(dbg["p1"][:], p_t[:])
                    nc.sync.dma_start(dbg["gate1"][:], gate[:])
                    nc.sync.dma_start(dbg["ar1"][:], ar_y[:])
                    nc.sync.dma_start(dbg["ar2"][:], ar_u[:])
                    nc.sync.dma_start(dbg["u1"][:], ut[:])
                    for n in range(NIMG):
                        nc.sync.dma_start(
                            dbg["x1"][n].rearrange("(ci p) s -> p ci s", p=P),
                            xt[:, n, :, :])

            conv_block(1, w1sb, dwt1, pk1, last=False)
            conv_block(2, w2sb, dwt2, pk2, last=True)

    nc.compile()
    return nc


def _pack_w(w):
    ws = np.sign(w.astype(np.float32))
    t = ws.reshape(2, P, CI, P, 3, 3)           # oi, o_lo, ci, c_lo, dy, dx
    t = t.transpose(3, 2, 4, 5, 0, 1)           # c_lo, ci, dy, dx, oi, o_lo
    return np.ascontiguousarray(t.reshape(P, CI, 9, 2, P)).astype(
        ml_dtypes.float8_e4m3)


def _pack_dw(dw):
    d = (dw.astype(np.float32) / 64.0).reshape(2, P, CI, P)  # oi,o_lo,ci,c_lo
    d = d.transpose(3, 2, 0, 1)                               # c_lo,ci,oi,o_lo
    hi = d.astype(ml_dtypes.bfloat16)
    lo = (d - hi.astype(np.float32)).astype(ml_dtypes.bfloat16)
    out = np.empty((P, CI, 2, 2, P), ml_dtypes.bfloat16)
    out[:, :, 0] = hi
    out[:, :, 1] = lo
    return out


def _pack_pk(w, a, g, b, dg, db):
    alpha = np.abs(w.astype(np.float32)).mean(axis=(1, 2, 3))
    fields = [alpha, a, g, b, dg, db]
    pk = np.empty((P, 6, CI), np.float32)
    for j, f in enumerate(fields):
        pk[:, j, :] = np.asarray(f, np.float32).reshape(CI, P).T
    return pk


def kernel(**inputs):
    key = ("dbg" if DEBUG else "std")
    if key not in _CACHE:
        _CACHE[key] = _build(debug=DEBUG)
    nc = _CACHE[key]

    x = np.asarray(inputs["x"], np.float32).reshape(64, 256, S)
    feed = {
        "w1sb": _pack_w(np.asarray(inputs["w1"])),
        "w2sb": _pack_w(np.asarray(inputs["w2"])),
        "dwt1": _pack_dw(np.asarray(inputs["dw1"])),
        "dwt2": _pack_dw(np.asarray(inputs["dw2"])),
        "pk1": _pack_pk(np.asarray(inputs["w1"]), inputs["a1"], inputs["g1"],
                        inputs["b1"], inputs["dg1"], inputs["db1"]),
        "pk2": _pack_pk(np.asarray(inputs["w2"]), inputs["a2"], inputs["g2"],
                        inputs["b2"], inputs["dg2"], inputs["db2"]),
    }
    in_maps = []
    for c in range(NCORES):
        m = dict(feed)
        m["x"] = np.ascontiguousarray(x[c * NIMG:(c + 1) * NIMG])
        in_maps.append(m)

    trace = bool(int(os.environ.get("BASS_KERNEL_TRACE", "0")))
    res = bass_utils.run_bass_kernel_spmd(
        nc, in_maps, core_ids=list(range(NCORES)), trace=trace)
    kernel.last_results = res

    out = np.concatenate([res.results[c]["out"] for c in range(NCORES)], axis=0)
    return out.reshape(64, 256, H, W)
